# revision 24
# baseline (speedup 1.0000x reference)
"""Multi-head attention TRN2 kernel: 8-core head-sharded tensor parallelism.

Full inputs in, full output out. Each core computes 2 of the 16 heads:
QKV projection (its column slice), flash-style attention, and a partial
out-projection against its row slice of Wo. Host sums the 8 partials and
adds (bv @ Wo + bo) once; the K bias is dropped entirely (it only adds a
per-query constant to the logits, which softmax cancels).

Per-core device program (identical SPMD; per-core weight slices via in_maps):
  projections: Q^T/K^T [128, T] (weights stationary) and V^T (x stationary,
           Wv moving) computed in 256-token units that are just-in-time
           interleaved into the attention loop's PE slack, so the scalar
           engine (exp) starts almost immediately and stays busy
  attention: per (batch, q-block, key-tile): both heads' score matmuls are
           row-tiled (partitions 0-63 / 64-127) into one 2-bank PSUM tile,
           one 1024-wide exp on ACT, ctx accumulation on PE against a
           ones-augmented V (the ones column emits softmax row-sums free)
  norm:    reciprocal of the row-sums, one broadcast matmul per head,
           multiply into ctx2t — split into DVE/PE work items so neither
           engine FIFO head-blocks
  out-proj: y[t, fo] = ctx2t-token-tile-stationary @ Wo_slice, interleaved,
           bf16 partials to DRAM

All SBUF tensors bf16 (fp32 PSUM accumulation).
"""
import sys

sys.path.insert(0, "/opt/trn_rl_repo")

from collections import deque
from contextlib import ExitStack

import numpy as np

import concourse.bass as bass
import concourse.tile as tile
from concourse import bacc, mybir
from concourse.bass_utils import run_bass_kernel_spmd

f32 = mybir.dt.float32
bf16 = mybir.dt.bfloat16
EXP = mybir.ActivationFunctionType.Exp

N_CORES = 8
B, S, F = 2, 2048, 1024
H = 16                 # heads total
DK = F // H            # 64
HPC = H // N_CORES     # 2 heads per core
CF = HPC * DK          # 128 = per-core slice of features
T = B * S              # 4096 tokens
TU = 512               # tokens per projection unit
NU = T // TU           # 8 projection units
NKT = S // 128         # 16 key tiles per sequence
NQB = S // 512         # 4 q-blocks per sequence
NC = F // 128          # 8 contraction chunks


def build_program():
    nc = bacc.Bacc("TRN2", target_bir_lowering=False, debug=False,
                   num_devices=N_CORES)

    xt_d = nc.dram_tensor("xT", [F, T], bf16, kind="ExternalInput").ap()
    wq_d = nc.dram_tensor("Wq", [F, CF], bf16, kind="ExternalInput").ap()
    wk_d = nc.dram_tensor("Wk", [F, CF], bf16, kind="ExternalInput").ap()
    wv_d = nc.dram_tensor("Wv", [F, CF], bf16, kind="ExternalInput").ap()
    bq_d = nc.dram_tensor("bq", [CF, 1], f32, kind="ExternalInput").ap()
    wo_d = nc.dram_tensor("Wo", [CF, F], bf16, kind="ExternalInput").ap()
    yp_d = nc.dram_tensor("yp", [T, F], bf16, kind="ExternalOutput").ap()

    with tile.TileContext(nc) as tc, ExitStack() as ctx:
        const = ctx.enter_context(tc.tile_pool(name="const", bufs=1))
        big = ctx.enter_context(tc.tile_pool(name="big", bufs=1))
        etp = ctx.enter_context(tc.tile_pool(name="etp", bufs=8))
        small = ctx.enter_context(tc.tile_pool(name="small", bufs=4))
        ypool = ctx.enter_context(tc.tile_pool(name="ypool", bufs=6))

        # ---- constants / inputs ----
        wq_sb = const.tile([128, NC, CF], bf16)
        nc.sync.dma_start(wq_sb, wq_d.rearrange("(a p) n -> p a n", p=128))
        # x input, one DMA per 512-token unit so unit j only waits its slice
        xall = const.tile([128, NU, NC, TU], bf16)
        nc.sync.dma_start(
            xall[:, 0, :, :],
            xt_d[:, 0:TU].rearrange("(a p) t -> p a t", p=128))
        wk_sb = const.tile([128, NC, CF], bf16)
        nc.sync.dma_start(wk_sb, wk_d.rearrange("(a p) n -> p a n", p=128))
        wv_sb = const.tile([128, NC, CF], bf16)
        nc.sync.dma_start(wv_sb, wv_d.rearrange("(a p) n -> p a n", p=128))
        bq_sb = const.tile([128, 1], f32)
        nc.sync.dma_start(bq_sb, bq_d)
        wo_sb = const.tile([128, F], bf16)
        nc.sync.dma_start(wo_sb, wo_d)
        for j in range(1, NU):
            nc.sync.dma_start(
                xall[:, j, :, :],
                xt_d[:, j * TU:(j + 1) * TU]
                .rearrange("(a p) t -> p a t", p=128))
        # ones row for the rowsum broadcast matmul
        ones_bf = const.tile([1, 64], bf16)
        nc.vector.memset(ones_bf, 1.0)

        # ---- persistent activations (all bf16) ----
        qt_sb = big.tile([128, T], bf16)       # [2 heads x 64 d, tokens]
        kt_sb = big.tile([128, T], bf16)
        # vaug per (b,kt): [tok-in-ktile, 130]:
        #   cols 0:64 = V head0, col 64 = ones, cols 65:129 = V head1,
        #   col 129 = ones.  head h ctx stationary = cols h*65 : h*65+65.
        vaug_sb = big.tile([128, B, NKT, 130], bf16)
        ctx2t_sb = big.tile([128, B, S], bf16)  # [2 heads x 64 d, b, tokens]

        nc.vector.memset(vaug_sb[:, :, :, 64:65], 1.0)
        nc.vector.memset(vaug_sb[:, :, :, 129:130], 1.0)

        # ---- PSUM pools ----
        sc_ps = ctx.enter_context(
            tc.tile_pool(name="sc_ps", bufs=2, space="PSUM"))   # 4 banks
        pc_ps = ctx.enter_context(
            tc.tile_pool(name="pc_ps", bufs=3, space="PSUM"))   # 3 banks
        aux_ps = ctx.enter_context(
            tc.tile_pool(name="aux_ps", bufs=1, space="PSUM"))  # 1 bank

        # ---- projection units (Q, K, V^T per 512 tokens) ----
        def emit_aq(j, pool, tag):
            """Q projection for tokens [j*512, (j+1)*512)."""
            pq = pool.tile([128, TU], f32, tag=tag, name=f"pq{j}")
            for c in range(NC):
                nc.tensor.matmul(pq, wq_sb[:, c, :], xall[:, j, c, :],
                                 start=(c == 0), stop=(c == NC - 1))
            sl = slice(j * TU, (j + 1) * TU)
            nc.vector.tensor_scalar_add(qt_sb[:, sl], pq, bq_sb)

        def emit_ak(j, pool, tag):
            """K projection for tokens [j*512, (j+1)*512)."""
            pk = pool.tile([128, TU], f32, tag=tag, name=f"pk{j}")
            for c in range(NC):
                nc.tensor.matmul(pk, wk_sb[:, c, :], xall[:, j, c, :],
                                 start=(c == 0), stop=(c == NC - 1))
            sl = slice(j * TU, (j + 1) * TU)
            nc.vector.tensor_copy(kt_sb[:, sl], pk)

        def emit_av(j, pool, tag):
            """V^T for the 4 key tiles in tokens [j*512, (j+1)*512)."""
            pv = pool.tile([128, 4, 128], f32, tag=tag, name=f"pv{j}")
            for i in range(4):
                for c in range(NC):
                    nc.tensor.matmul(
                        pv[:, i, :],
                        xall[:, j, c, i * 128:(i + 1) * 128],
                        wv_sb[:, c, :],
                        start=(i == 0 and c == 0),
                        stop=(i == 3 and c == NC - 1))
            bj = j // 4
            kt0 = (j % 4) * 4
            nc.vector.tensor_copy(
                vaug_sb[:, bj, kt0:kt0 + 4, 0:64], pv[:, :, 0:64])
            nc.vector.tensor_copy(
                vaug_sb[:, bj, kt0:kt0 + 4, 65:129], pv[:, :, 64:128])

        # serial head: just enough projections for the first attention steps,
        # run in parallel across the (still empty) pc ring
        emit_aq(0, pc_ps, "pc")
        emit_ak(0, pc_ps, "pc")
        emit_av(0, pc_ps, "pc")

        # remaining units, just-in-time: (deadline_iteration, kind, j)
        # b0 tight (data-dependent), b1 spread gently through b0's steps
        jit = deque(sorted([
            (2, "k", 1), (4, "v", 1), (6, "k", 2), (8, "v", 2),
            (10, "k", 3), (12, "v", 3), (14, "q", 1),
            (16, "q", 4), (18, "k", 4), (20, "v", 4), (22, "q", 5),
            (24, "k", 5), (26, "v", 5), (28, "q", 2), (30, "k", 6),
            (32, "v", 6), (34, "q", 6), (36, "k", 7), (38, "v", 7),
            (40, "q", 3), (42, "q", 7),
        ], key=lambda t: t[0]))

        # ---- attention steps: one per (b, qb, kt) ----
        steps = []
        for b in range(B):
            for qb in range(NQB):
                for kt in range(NKT):
                    steps.append((b, qb, kt))

        score_ps = {}
        exp_sb = {}
        ctx_ps = {}
        work = deque()   # deferred norm / out-proj units

        def emit_scores(i):
            b, qb, kt = steps[i]
            pss = sc_ps.tile([128, 2, 512], f32, tag="sc", name=f"sc{i}")
            for h in range(2):
                nc.tensor.matmul(
                    pss[:, h, :],
                    kt_sb[h * 64:(h + 1) * 64,
                          b * S + kt * 128:b * S + (kt + 1) * 128],
                    qt_sb[h * 64:(h + 1) * 64,
                          b * S + qb * 512:b * S + (qb + 1) * 512],
                    start=True, stop=True)
            score_ps[i] = pss

        def emit_exp(i):
            et = etp.tile([128, 2, 512], bf16, tag="et", name=f"et{i}")
            nc.scalar.activation(et, score_ps.pop(i), EXP)
            exp_sb[i] = et

        def emit_ctx(i):
            b, qb, kt = steps[i]
            if kt == 0:
                ctx_ps[(b, qb, 0)] = pc_ps.tile(
                    [65, 512], f32, tag="pc", name=f"pc{i}h0")
                ctx_ps[(b, qb, 1)] = pc_ps.tile(
                    [65, 512], f32, tag="pc", name=f"pc{i}h1")
            et = exp_sb.pop(i)
            for h in range(2):
                nc.tensor.matmul(
                    ctx_ps[(b, qb, h)],
                    vaug_sb[:, b, kt, h * 65:h * 65 + 65],
                    et[:, h, :],
                    start=(kt == 0), stop=(kt == NKT - 1))
            if kt == NKT - 1:
                work.append(("norm_dve", b, qb))

        norm_state = {}

        def emit_norm_dve(b, qb):
            """DVE half: rowsum copies + reciprocal + bf16 cast."""
            pc0 = ctx_ps[(b, qb, 0)]
            pc1 = ctx_ps[(b, qb, 1)]
            rs = small.tile([1, 2, 512], f32, tag="rs", name=f"rs{b}{qb}")
            # plain copies: custom-DVE ops don't partition-shift their input
            nc.vector.tensor_copy(rs[0:1, 0, :], pc0[64:65, :])
            nc.vector.tensor_copy(rs[0:1, 1, :], pc1[64:65, :])
            rcp = small.tile([1, 2, 512], f32, tag="rcp", name=f"rcp{b}{qb}")
            nc.vector.reciprocal_approx_fast(rcp, rs)
            rcpb = small.tile([1, 2, 512], bf16, tag="rcpb",
                              name=f"rcpb{b}{qb}")
            nc.vector.tensor_copy(rcpb, rcp)
            norm_state[(b, qb)] = rcpb
            work.append(("norm_pe", b, qb))

        def emit_norm_pe(b, qb):
            """PE half: broadcast matmuls + normalize into ctx2t."""
            pc0 = ctx_ps.pop((b, qb, 0))
            pc1 = ctx_ps.pop((b, qb, 1))
            rcpb = norm_state.pop((b, qb))
            dst = ctx2t_sb[:, b, qb * 512:(qb + 1) * 512]
            pcs = (pc0, pc1)
            for h in range(2):
                pb = aux_ps.tile([64, 512], f32, tag="aux",
                                 name=f"pb{b}{qb}{h}")
                nc.tensor.matmul(pb, ones_bf, rcpb[0:1, h, :],
                                 start=True, stop=True)
                pbs = small.tile([64, 512], bf16, tag="pbs",
                                 name=f"pbs{b}{qb}{h}")
                nc.vector.tensor_copy(pbs, pb)
                nc.vector.tensor_mul(dst[h * 64:(h + 1) * 64, :],
                                     pcs[h][0:64, :], pbs)
            for tt in range(4):
                work.append(("op", b, qb, tt))

        def emit_outproj(b, qb, tt, pool=None, tag=None):
            pool = pool or aux_ps
            tag = tag or "aux"
            tok0 = qb * 512 + tt * 128
            ysb = ypool.tile([128, 1024], bf16, tag="ysb",
                             name=f"ysb{b}{qb}{tt}")
            for wh in range(2):
                py = pool.tile([128, 512], f32, tag=tag,
                               name=f"py{b}{qb}{tt}{wh}")
                nc.tensor.matmul(
                    py, ctx2t_sb[:, b, tok0:tok0 + 128],
                    wo_sb[:, wh * 512:(wh + 1) * 512],
                    start=True, stop=True)
                nc.vector.tensor_copy(ysb[:, wh * 512:(wh + 1) * 512], py)
            nc.sync.dma_start(
                yp_d[b * S + tok0:b * S + tok0 + 128, :], ysb)

        def drain_work(n=1):
            for _ in range(n):
                if not work:
                    return
                item = work.popleft()
                if item[0] == "norm_dve":
                    emit_norm_dve(item[1], item[2])
                elif item[0] == "norm_pe":
                    emit_norm_pe(item[1], item[2])
                else:
                    emit_outproj(item[1], item[2], item[3])

        def drain_jit(i):
            while jit and jit[0][0] <= i:
                _, kind, j = jit.popleft()
                if kind == "q":
                    emit_aq(j, aux_ps, "aux")
                elif kind == "k":
                    emit_ak(j, aux_ps, "aux")
                else:
                    emit_av(j, aux_ps, "aux")

        emit_scores(0)
        emit_scores(1)
        emit_exp(0)
        for i in range(2, len(steps)):
            drain_jit(i)
            emit_scores(i)
            emit_exp(i - 1)
            emit_ctx(i - 2)
            if i >= 110:
                drain_work(1)
            elif i % 3 == 0:
                drain_work(1)
        emit_exp(len(steps) - 1)
        emit_ctx(len(steps) - 2)
        emit_ctx(len(steps) - 1)
        # final drain: norms through the normal path, then the remaining
        # out-projections pipelined through the (now empty) 3-slot pc ring
        while work and work[0][0] != "op":
            drain_work(1)
        while work:
            item = work.popleft()
            if item[0] == "op":
                emit_outproj(item[1], item[2], item[3], pc_ps, "pc")
            elif item[0] == "norm_dve":
                emit_norm_dve(item[1], item[2])
            else:
                emit_norm_pe(item[1], item[2])

    nc.compile()
    return nc


_NC = None


def _to_bf16(a):
    import ml_dtypes
    return np.asarray(a, dtype=np.float32).astype(ml_dtypes.bfloat16)


def make_in_maps(inputs):
    """Build the 8 per-core input maps from full-precision inputs."""
    x = np.asarray(inputs["x"], dtype=np.float32)
    sc = 1.0 / np.sqrt(np.float32(DK))
    xT = np.ascontiguousarray(x.reshape(T, F).T)
    xT16 = _to_bf16(xT)
    in_maps = []
    for c in range(N_CORES):
        sl = slice(c * CF, (c + 1) * CF)
        in_maps.append({
            "xT": xT16,
            "Wq": _to_bf16(np.asarray(inputs["Wq"])[:, sl] * sc),
            "Wk": _to_bf16(np.asarray(inputs["Wk"])[:, sl]),
            "Wv": _to_bf16(np.asarray(inputs["Wv"])[:, sl]),
            "bq": np.ascontiguousarray(
                (np.asarray(inputs["bq"])[sl] * sc)
                .astype(np.float32).reshape(CF, 1)),
            "Wo": _to_bf16(np.asarray(inputs["Wo"])[sl, :]),
        })
    return in_maps


def combine_outputs(results, inputs):
    """Sum per-core bf16 partials, add host-side bias terms."""
    y = np.zeros((T, F), dtype=np.float64)
    for c in range(N_CORES):
        y += np.asarray(results[c]["yp"], dtype=np.float64)
    bo = np.asarray(inputs["bo"], dtype=np.float64)
    bv = np.asarray(inputs["bv"], dtype=np.float64)
    Wo = np.asarray(inputs["Wo"], dtype=np.float64)
    y += bo + bv @ Wo
    return y.astype(np.float32).reshape(B, S, F)


def kernel(x, Wq, bq, Wk, bk, Wv, bv, Wo, bo):
    global _NC
    if _NC is None:
        _NC = build_program()
    inputs = {"x": x, "Wq": Wq, "bq": bq, "Wk": Wk, "bk": bk,
              "Wv": Wv, "bv": bv, "Wo": Wo, "bo": bo}
    in_maps = make_in_maps(inputs)
    res = run_bass_kernel_spmd(_NC, in_maps, list(range(N_CORES)))
    return combine_outputs(res.results, inputs)
